# revision 22
# baseline (speedup 1.0000x reference)
"""GroupWiseLinear Trainium2 kernel (paired-stationary bins, ramped schedule).

out[b, c] = dot(W[0, c, :], x[b, group_of[c], :]) + bias[0, c], then a final
class-permutation gather, for two independent branches (co / cl).

Sharding: 8 cores = 2 branches x 4 shards.  Whole groups are assigned to
shards by LPT bin packing (~1024 classes each, no group straddles a core).
One uniform SPMD program runs on all cores; per-position bin widths are the
max over cores (zero-padded W elsewhere).

Device scheme per core:
  - Group segments (split at 200 cols) are paired so pair widths approach a
    target; each pair is a "bin": the two groups' x^T chunks side by side
    form a [128, 128] matmul stationary (batches 0-63 -> PSUM partitions
    0-63 for half A, 64-127 for half B).  The bin's W columns stream
    unpadded through one matmul per 128-deep K chunk (4), accumulating
    [128, N_bin] fp32 in PSUM.
  - Every output column is valid in exactly one half; the host picks the
    half, adds bias, and applies the final class permutation.
  - Schedule: bins are emitted smallest-first ramp -> big -> tiny tail.
    Each slice loads ONE combined [x-part | W-part] DMA (SP queue), its
    PSUM bank is copied to SBUF as bf16 (DVE) and output DMAs cover two
    slices each (9 DMA instructions total keeps the shared HWDGE
    descriptor generator off the critical path; tiny last slices keep the
    post-load tail short).

HBM traffic per core ~2.6 MB (bf16 wire, fp32 PSUM accumulate) vs ~4.5 MB
for the slot-padded baseline.
"""

import ml_dtypes
import numpy as np

import concourse.bacc as bacc
import concourse.tile as tile
from concourse import mybir
from concourse.bass_utils import run_bass_kernel_spmd

B = 64          # batch
H = 512         # hidden
NC_CLS = 4096   # classes per branch
KC = H // 128   # contraction chunks
PSUM_W = 512    # PSUM bank width in fp32 columns
MAX_SEG = 200   # split segments wider than this
PAIR_TGT = 144  # aim pair widths here

_cache = {}


def _segments(go):
    segs = []
    i = 0
    n = len(go)
    while i < n:
        g = int(go[i])
        j = i
        while j < n and go[j] == g:
            j += 1
        segs.append((g, i, j - i))
        i = j
    return segs


def _lpt_shard(segs, nshards):
    loads = [0] * nshards
    shards = [[] for _ in range(nshards)]
    for s in sorted(segs, key=lambda t: -t[2]):
        k = min(range(nshards), key=lambda i: loads[i])
        shards[k].append(s)
        loads[k] += s[2]
    return shards


def _make_bins(segs):
    """Split wide segments, then pair seeking PAIR_TGT total width."""
    items = []
    for g, s, w in segs:
        for o in range(0, w, MAX_SEG):
            items.append((g, s + o, min(MAX_SEG, w - o)))
    items.sort(key=lambda t: -t[2])
    bins = []
    while items:
        a = items.pop(0)
        if not items:
            bins.append([a])
            break
        want = PAIR_TGT - a[2]
        bi = min(range(len(items)), key=lambda i: abs(items[i][2] - want))
        if a[2] + items[bi][2] > PSUM_W:
            bins.append([a])
        else:
            bins.append([a, items.pop(bi)])
    bins.sort(key=lambda bn: -sum(p[2] for p in bn))
    return bins


def _plan(co_group_of, cl_group_of):
    gos = (np.asarray(co_group_of).astype(np.int64),
           np.asarray(cl_group_of).astype(np.int64))
    core_bins = []
    for bi in range(2):
        for sh in _lpt_shard(_segments(gos[bi]), 4):
            core_bins.append(_make_bins(sh))
    nbins = max(len(cb) for cb in core_bins)
    for cb in core_bins:
        while len(cb) < nbins:
            cb.append([])
    prof_desc = [
        max(sum(p[2] for p in cb[j]) for cb in core_bins) for j in range(nbins)
    ]
    prof_desc = [(max(w, 8) + 7) // 8 * 8 for w in prof_desc]

    # Emission order: one mid bin first (PE warm-up on a small load), then
    # descending sizes, tiny bins last.
    mid = min(nbins - 1, (nbins + 1) // 2)
    order = [mid] + [j for j in range(nbins) if j != mid]
    profile = [prof_desc[j] for j in order]
    for cb in core_bins:
        cb[:] = [cb[j] for j in order]

    # Slices: consecutive bins, each slice's width <= PSUM_W, sized so the
    # stream ramps: first slice 1 bin, then 2 bins each, tail 1-bin slices.
    sizes = [1]
    rem = nbins - 1
    while rem > 2:
        take = 2 if rem > 4 else 1
        sizes.append(take)
        rem -= take
    while rem:
        sizes.append(1)
        rem -= 1
    bounds = [0]
    for s in sizes:
        bounds.append(bounds[-1] + s)

    # PSUM banks: greedy <=PSUM_W within each slice.
    tiles = []
    for s in range(len(sizes)):
        b0, b1 = bounds[s], bounds[s + 1]
        t0, acc = b0, 0
        for j in range(b0, b1):
            if acc + profile[j] > PSUM_W:
                tiles.append((s, t0, j))
                t0, acc = j, 0
            acc += profile[j]
        tiles.append((s, t0, b1))

    # Last bin position is a lone item (or empty) on every core -> its
    # stationary only needs 64 columns (M=64), halving its xt block.
    single_last = all(len(cb[nbins - 1]) <= 1 for cb in core_bins)

    # Output DMA groups: pairs of consecutive slices, except one merged
    # group for the small tail slices so only ONE out-DMA chain trails the
    # final load.
    nsl = len(sizes)
    tail0 = nsl
    tailw = 0
    while tail0 > 1 and tailw + profile[bounds[tail0 - 1]] <= PSUM_W // 2:
        tailw += sum(profile[bounds[tail0 - 1] : bounds[tail0]])
        tail0 -= 1
    ogroups = []
    s = 0
    while s < tail0:
        s1 = min(s + 2, tail0)
        ogroups.append((s, s1))
        s = s1
    if tail0 < nsl:
        ogroups.append((tail0, nsl))
    return core_bins, profile, bounds, tiles, ogroups, single_last


def _xw_layout(profile, bounds, single_last):
    """Per-slice (offset, per-bin x offsets, x cols, w cols) in the combined
    xw buffer; the last bin's stationary is 64 wide when single_last."""
    nbins = len(profile)
    nsl = len(bounds) - 1
    prefn = np.concatenate([[0], np.cumsum(profile)]).astype(int)
    xblk = [KC * 128] * nbins
    if single_last:
        xblk[nbins - 1] = KC * 64
    soff = []
    off = 0
    for s in range(nsl):
        b0, b1 = bounds[s], bounds[s + 1]
        xoffs = []
        xc = 0
        for j in range(b0, b1):
            xoffs.append(xc)
            xc += xblk[j]
        wc = KC * int(prefn[b1] - prefn[b0])
        soff.append((off, xoffs, xc, wc))
        off += xc + wc
    return soff, off, prefn


def _program(profile, bounds, tiles, ogroups, single_last,
             dt=mybir.dt.bfloat16):
    nbins = len(profile)
    nsl = len(bounds) - 1
    soff, tot_xw, prefn = _xw_layout(profile, bounds, single_last)
    totn = int(prefn[-1])

    nc = bacc.Bacc("TRN2", target_bir_lowering=False, debug=False, num_devices=8)
    xw_d = nc.dram_tensor("xw", [128, tot_xw], dt, kind="ExternalInput")
    o_d = nc.dram_tensor("o", [128, totn], dt, kind="ExternalOutput")

    with tile.TileContext(nc) as tc:
        with (
            tc.tile_pool(name="xp", bufs=nsl) as xp,
            tc.tile_pool(name="op", bufs=1) as op,
            tc.tile_pool(name="ps", bufs=1, space="PSUM") as ps,
        ):
            stiles = {}
            for s in range(nsl):
                o0, _, xc, wc = soff[s]
                xws = xp.tile([128, xc + wc], dt)
                nc.sync.dma_start(xws[:], xw_d[:, o0 : o0 + xc + wc])
                stiles[s] = xws

            # one SBUF output tile per output group; PSUM copies land in it
            grp_of_slice = {}
            for gi, (g0, g1) in enumerate(ogroups):
                for s in range(g0, g1):
                    grp_of_slice[s] = gi
            gtiles = {}
            for gi, (g0, g1) in enumerate(ogroups):
                gw = int(prefn[bounds[g1]] - prefn[bounds[g0]])
                gtiles[gi] = op.tile([128, gw], dt, name=f"og{gi}")

            for ti, (s, tb0, tb1) in enumerate(tiles):
                b0, b1 = bounds[s], bounds[s + 1]
                o0, xoffs, xc, wc = soff[s]
                xws = stiles[s]
                tw = int(prefn[tb1] - prefn[tb0])
                acc = ps.tile([128, tw], mybir.dt.float32, name=f"ps{ti}")
                for j in range(tb0, tb1):
                    n = profile[j]
                    m = 64 if (single_last and j == nbins - 1) else 128
                    xoff = xoffs[j - b0]
                    woff = xc + KC * int(prefn[j] - prefn[b0])
                    aoff = int(prefn[j] - prefn[tb0])
                    for k in range(KC):
                        nc.tensor.matmul(
                            acc[0:m, aoff : aoff + n],
                            xws[:, xoff + k * m : xoff + (k + 1) * m],
                            xws[:, woff + k * n : woff + (k + 1) * n],
                            start=(k == 0),
                            stop=(k == KC - 1),
                        )
                gi = grp_of_slice[s]
                g0, g1 = ogroups[gi]
                got = gtiles[gi]
                goff = int(prefn[tb0] - prefn[bounds[g0]])
                nc.vector.tensor_copy(got[:, goff : goff + tw], acc[:])
                if tb1 == bounds[g1]:  # group complete -> single out DMA
                    # final group rides SP: shorter decode + DGE delay
                    eng = nc.sync if gi == len(ogroups) - 1 else nc.scalar
                    eng.dma_start(
                        o_d[:, int(prefn[bounds[g0]]) : int(prefn[bounds[g1]])],
                        got[:],
                    )

    nc.compile()
    return nc


def _host_prep(x, W, bins, profile, bounds, single_last, goff,
               dt=ml_dtypes.bfloat16):
    nbins = len(profile)
    soff, tot_xw, prefn = _xw_layout(profile, bounds, single_last)
    xw = np.zeros((128, tot_xw), dt)

    classes, ocols, halves = [], [], []
    xg_cache = {}

    def xgT(g):
        if g not in xg_cache:
            xg_cache[g] = np.ascontiguousarray(
                x[:, goff + g, :].astype(np.float32)
                .reshape(B, KC, 128).transpose(2, 1, 0)
            ).astype(dt)  # [128, KC, 64]
        return xg_cache[g]

    sl_of = np.searchsorted(bounds, np.arange(nbins), side="right") - 1
    for j, bn in enumerate(bins):
        s = int(sl_of[j])
        b0 = bounds[s]
        o0, xoffs, xc, wc = soff[s]
        n = profile[j]
        m = 64 if (single_last and j == nbins - 1) else 128
        coff = 0
        for hidx, (g, cs, w) in enumerate(bn):
            xs = xgT(g)
            for k in range(KC):
                c0 = o0 + xoffs[j - b0] + k * m + hidx * 64
                xw[:, c0 : c0 + 64] = xs[:, k, :]
            wTk = W[cs : cs + w].astype(np.float32).reshape(w, KC, 128)
            for k in range(KC):
                wcol = o0 + xc + KC * int(prefn[j] - prefn[b0]) + k * n + coff
                xw[:, wcol : wcol + w] = wTk[:, k, :].T
            classes.append(np.arange(cs, cs + w))
            ocols.append(int(prefn[j]) + coff + np.arange(w))
            halves.append(np.full(w, hidx))
            coff += w
    return ({"xw": xw},
            np.concatenate(classes), np.concatenate(ocols),
            np.concatenate(halves))


def kernel(x, co_W, cl_W, co_b, cl_b, co_group_of, cl_group_of, co_index,
           cl_index, group_len):
    x = np.asarray(x, np.float32)
    G = int(group_len)
    core_bins, profile, bounds, tiles, ogroups, single_last = _plan(
        co_group_of, cl_group_of
    )

    key = ("v5", tuple(profile), tuple(bounds), tuple(tiles), tuple(ogroups),
           single_last)
    if key not in _cache:
        _cache.clear()
        _cache[key] = _program(profile, bounds, tiles, ogroups, single_last)
    nc = _cache[key]

    Ws = (np.asarray(co_W, np.float32)[0], np.asarray(cl_W, np.float32)[0])
    bs = (np.asarray(co_b, np.float32)[0], np.asarray(cl_b, np.float32)[0])
    in_maps, colmaps = [], []
    for k in range(8):
        bi = k // 4
        im, classes, ocols, halves = _host_prep(
            x, Ws[bi], core_bins[k], profile, bounds, single_last, bi * G
        )
        in_maps.append(im)
        colmaps.append((classes, ocols, halves))

    res = run_bass_kernel_spmd(nc, in_maps, list(range(8)))

    outs = []
    for bi, index in ((0, co_index), (1, cl_index)):
        full = np.empty((B, NC_CLS), np.float32)
        for q in range(4):
            core = bi * 4 + q
            classes, ocols, halves = colmaps[core]
            oarr = np.asarray(res.results[core]["o"], np.float32)
            o3 = oarr.reshape(2, B, oarr.shape[1])
            full[:, classes] = o3[halves, :, ocols].T
        full += bs[bi][None, :]
        outs.append(full[:, np.asarray(index).astype(np.int64)])
    return outs[0], outs[1]


# revision 23
# speedup vs baseline: 1.0027x; 1.0027x over previous
"""GroupWiseLinear Trainium2 kernel (paired-stationary bins, ramped schedule).

out[b, c] = dot(W[0, c, :], x[b, group_of[c], :]) + bias[0, c], then a final
class-permutation gather, for two independent branches (co / cl).

Sharding: 8 cores = 2 branches x 4 shards.  Whole groups are assigned to
shards by LPT bin packing (~1024 classes each, no group straddles a core).
One uniform SPMD program runs on all cores; per-position bin widths are the
max over cores (zero-padded W elsewhere).

Device scheme per core:
  - Group segments (split at 200 cols) are paired so pair widths approach a
    target; each pair is a "bin": the two groups' x^T chunks side by side
    form a [128, 128] matmul stationary (batches 0-63 -> PSUM partitions
    0-63 for half A, 64-127 for half B).  The bin's W columns stream
    unpadded through one matmul per 128-deep K chunk (4), accumulating
    [128, N_bin] fp32 in PSUM.
  - Every output column is valid in exactly one half; the host picks the
    half, adds bias, and applies the final class permutation.
  - Schedule: bins are emitted smallest-first ramp -> big -> tiny tail.
    Each slice loads ONE combined [x-part | W-part] DMA (SP queue), its
    PSUM bank is copied to SBUF as bf16 (DVE) and output DMAs cover two
    slices each (9 DMA instructions total keeps the shared HWDGE
    descriptor generator off the critical path; tiny last slices keep the
    post-load tail short).

HBM traffic per core ~2.6 MB (bf16 wire, fp32 PSUM accumulate) vs ~4.5 MB
for the slot-padded baseline.
"""

import ml_dtypes
import numpy as np

import concourse.bacc as bacc
import concourse.tile as tile
from concourse import mybir
from concourse.bass_utils import run_bass_kernel_spmd

B = 64          # batch
H = 512         # hidden
NC_CLS = 4096   # classes per branch
KC = H // 128   # contraction chunks
PSUM_W = 512    # PSUM bank width in fp32 columns
MAX_SEG = 200   # split segments wider than this
PAIR_TGT = 152  # aim pair widths here

_cache = {}


def _segments(go):
    segs = []
    i = 0
    n = len(go)
    while i < n:
        g = int(go[i])
        j = i
        while j < n and go[j] == g:
            j += 1
        segs.append((g, i, j - i))
        i = j
    return segs


def _lpt_shard(segs, nshards):
    loads = [0] * nshards
    shards = [[] for _ in range(nshards)]
    for s in sorted(segs, key=lambda t: -t[2]):
        k = min(range(nshards), key=lambda i: loads[i])
        shards[k].append(s)
        loads[k] += s[2]
    return shards


def _make_bins(segs):
    """Split wide segments, then pair seeking PAIR_TGT total width."""
    items = []
    for g, s, w in segs:
        for o in range(0, w, MAX_SEG):
            items.append((g, s + o, min(MAX_SEG, w - o)))
    items.sort(key=lambda t: -t[2])
    bins = []
    while items:
        a = items.pop(0)
        if not items:
            bins.append([a])
            break
        want = PAIR_TGT - a[2]
        bi = min(range(len(items)), key=lambda i: abs(items[i][2] - want))
        if a[2] + items[bi][2] > PSUM_W:
            bins.append([a])
        else:
            bins.append([a, items.pop(bi)])
    bins.sort(key=lambda bn: -sum(p[2] for p in bn))
    return bins


def _plan(co_group_of, cl_group_of):
    gos = (np.asarray(co_group_of).astype(np.int64),
           np.asarray(cl_group_of).astype(np.int64))
    core_bins = []
    for bi in range(2):
        for sh in _lpt_shard(_segments(gos[bi]), 4):
            core_bins.append(_make_bins(sh))
    nbins = max(len(cb) for cb in core_bins)
    for cb in core_bins:
        while len(cb) < nbins:
            cb.append([])
    prof_desc = [
        max(sum(p[2] for p in cb[j]) for cb in core_bins) for j in range(nbins)
    ]
    prof_desc = [(max(w, 8) + 7) // 8 * 8 for w in prof_desc]

    # Emission order: one mid bin first (PE warm-up on a small load), then
    # descending sizes, tiny bins last.
    mid = min(nbins - 1, (nbins + 1) // 2)
    order = [mid] + [j for j in range(nbins) if j != mid]
    profile = [prof_desc[j] for j in order]
    for cb in core_bins:
        cb[:] = [cb[j] for j in order]

    # Slices: consecutive bins, each slice's width <= PSUM_W, sized so the
    # stream ramps: first slice 1 bin, then 2 bins each, tail 1-bin slices.
    sizes = [1]
    rem = nbins - 1
    while rem > 2:
        take = 2 if rem > 4 else 1
        sizes.append(take)
        rem -= take
    while rem:
        sizes.append(1)
        rem -= 1
    bounds = [0]
    for s in sizes:
        bounds.append(bounds[-1] + s)

    # PSUM banks: greedy <=PSUM_W within each slice.
    tiles = []
    for s in range(len(sizes)):
        b0, b1 = bounds[s], bounds[s + 1]
        t0, acc = b0, 0
        for j in range(b0, b1):
            if acc + profile[j] > PSUM_W:
                tiles.append((s, t0, j))
                t0, acc = j, 0
            acc += profile[j]
        tiles.append((s, t0, b1))

    # Last bin position is a lone item (or empty) on every core -> its
    # stationary only needs 64 columns (M=64), halving its xt block.
    single_last = all(len(cb[nbins - 1]) <= 1 for cb in core_bins)

    # Output DMA groups: pairs of consecutive slices, except one merged
    # group for the small tail slices so only ONE out-DMA chain trails the
    # final load.
    nsl = len(sizes)
    tail0 = nsl
    tailw = 0
    while tail0 > 1 and tailw + profile[bounds[tail0 - 1]] <= PSUM_W // 2:
        tailw += sum(profile[bounds[tail0 - 1] : bounds[tail0]])
        tail0 -= 1
    ogroups = []
    s = 0
    while s < tail0:
        s1 = min(s + 2, tail0)
        ogroups.append((s, s1))
        s = s1
    if tail0 < nsl:
        ogroups.append((tail0, nsl))
    return core_bins, profile, bounds, tiles, ogroups, single_last


def _xw_layout(profile, bounds, single_last):
    """Per-slice (offset, per-bin x offsets, x cols, w cols) in the combined
    xw buffer; the last bin's stationary is 64 wide when single_last."""
    nbins = len(profile)
    nsl = len(bounds) - 1
    prefn = np.concatenate([[0], np.cumsum(profile)]).astype(int)
    xblk = [KC * 128] * nbins
    if single_last:
        xblk[nbins - 1] = KC * 64
    soff = []
    off = 0
    for s in range(nsl):
        b0, b1 = bounds[s], bounds[s + 1]
        xoffs = []
        xc = 0
        for j in range(b0, b1):
            xoffs.append(xc)
            xc += xblk[j]
        wc = KC * int(prefn[b1] - prefn[b0])
        soff.append((off, xoffs, xc, wc))
        off += xc + wc
    return soff, off, prefn


def _program(profile, bounds, tiles, ogroups, single_last,
             dt=mybir.dt.bfloat16):
    nbins = len(profile)
    nsl = len(bounds) - 1
    soff, tot_xw, prefn = _xw_layout(profile, bounds, single_last)
    totn = int(prefn[-1])

    nc = bacc.Bacc("TRN2", target_bir_lowering=False, debug=False, num_devices=8)
    xw_d = nc.dram_tensor("xw", [128, tot_xw], dt, kind="ExternalInput")
    o_d = nc.dram_tensor("o", [128, totn], dt, kind="ExternalOutput")

    with tile.TileContext(nc) as tc:
        with (
            tc.tile_pool(name="xp", bufs=nsl) as xp,
            tc.tile_pool(name="op", bufs=1) as op,
            tc.tile_pool(name="ps", bufs=1, space="PSUM") as ps,
        ):
            stiles = {}
            for s in range(nsl):
                o0, _, xc, wc = soff[s]
                xws = xp.tile([128, xc + wc], dt)
                nc.sync.dma_start(xws[:], xw_d[:, o0 : o0 + xc + wc])
                stiles[s] = xws

            # one SBUF output tile per output group; PSUM copies land in it
            grp_of_slice = {}
            for gi, (g0, g1) in enumerate(ogroups):
                for s in range(g0, g1):
                    grp_of_slice[s] = gi
            gtiles = {}
            for gi, (g0, g1) in enumerate(ogroups):
                gw = int(prefn[bounds[g1]] - prefn[bounds[g0]])
                gtiles[gi] = op.tile([128, gw], dt, name=f"og{gi}")

            for ti, (s, tb0, tb1) in enumerate(tiles):
                b0, b1 = bounds[s], bounds[s + 1]
                o0, xoffs, xc, wc = soff[s]
                xws = stiles[s]
                tw = int(prefn[tb1] - prefn[tb0])
                acc = ps.tile([128, tw], mybir.dt.float32, name=f"ps{ti}")
                for j in range(tb0, tb1):
                    n = profile[j]
                    m = 64 if (single_last and j == nbins - 1) else 128
                    xoff = xoffs[j - b0]
                    woff = xc + KC * int(prefn[j] - prefn[b0])
                    aoff = int(prefn[j] - prefn[tb0])
                    for k in range(KC):
                        nc.tensor.matmul(
                            acc[0:m, aoff : aoff + n],
                            xws[:, xoff + k * m : xoff + (k + 1) * m],
                            xws[:, woff + k * n : woff + (k + 1) * n],
                            start=(k == 0),
                            stop=(k == KC - 1),
                        )
                gi = grp_of_slice[s]
                g0, g1 = ogroups[gi]
                got = gtiles[gi]
                goff = int(prefn[tb0] - prefn[bounds[g0]])
                nc.vector.tensor_copy(got[:, goff : goff + tw], acc[:])
                if tb1 == bounds[g1]:  # group complete -> single out DMA
                    # final group rides SP: shorter decode + DGE delay
                    eng = nc.sync if gi == len(ogroups) - 1 else nc.scalar
                    eng.dma_start(
                        o_d[:, int(prefn[bounds[g0]]) : int(prefn[bounds[g1]])],
                        got[:],
                    )

    nc.compile()
    return nc


def _host_prep(x, W, bins, profile, bounds, single_last, goff,
               dt=ml_dtypes.bfloat16):
    nbins = len(profile)
    soff, tot_xw, prefn = _xw_layout(profile, bounds, single_last)
    xw = np.zeros((128, tot_xw), dt)

    classes, ocols, halves = [], [], []
    xg_cache = {}

    def xgT(g):
        if g not in xg_cache:
            xg_cache[g] = np.ascontiguousarray(
                x[:, goff + g, :].astype(np.float32)
                .reshape(B, KC, 128).transpose(2, 1, 0)
            ).astype(dt)  # [128, KC, 64]
        return xg_cache[g]

    sl_of = np.searchsorted(bounds, np.arange(nbins), side="right") - 1
    for j, bn in enumerate(bins):
        s = int(sl_of[j])
        b0 = bounds[s]
        o0, xoffs, xc, wc = soff[s]
        n = profile[j]
        m = 64 if (single_last and j == nbins - 1) else 128
        coff = 0
        for hidx, (g, cs, w) in enumerate(bn):
            xs = xgT(g)
            for k in range(KC):
                c0 = o0 + xoffs[j - b0] + k * m + hidx * 64
                xw[:, c0 : c0 + 64] = xs[:, k, :]
            wTk = W[cs : cs + w].astype(np.float32).reshape(w, KC, 128)
            for k in range(KC):
                wcol = o0 + xc + KC * int(prefn[j] - prefn[b0]) + k * n + coff
                xw[:, wcol : wcol + w] = wTk[:, k, :].T
            classes.append(np.arange(cs, cs + w))
            ocols.append(int(prefn[j]) + coff + np.arange(w))
            halves.append(np.full(w, hidx))
            coff += w
    return ({"xw": xw},
            np.concatenate(classes), np.concatenate(ocols),
            np.concatenate(halves))


def kernel(x, co_W, cl_W, co_b, cl_b, co_group_of, cl_group_of, co_index,
           cl_index, group_len):
    x = np.asarray(x, np.float32)
    G = int(group_len)
    core_bins, profile, bounds, tiles, ogroups, single_last = _plan(
        co_group_of, cl_group_of
    )

    key = ("v5", tuple(profile), tuple(bounds), tuple(tiles), tuple(ogroups),
           single_last)
    if key not in _cache:
        _cache.clear()
        _cache[key] = _program(profile, bounds, tiles, ogroups, single_last)
    nc = _cache[key]

    Ws = (np.asarray(co_W, np.float32)[0], np.asarray(cl_W, np.float32)[0])
    bs = (np.asarray(co_b, np.float32)[0], np.asarray(cl_b, np.float32)[0])
    in_maps, colmaps = [], []
    for k in range(8):
        bi = k // 4
        im, classes, ocols, halves = _host_prep(
            x, Ws[bi], core_bins[k], profile, bounds, single_last, bi * G
        )
        in_maps.append(im)
        colmaps.append((classes, ocols, halves))

    res = run_bass_kernel_spmd(nc, in_maps, list(range(8)))

    outs = []
    for bi, index in ((0, co_index), (1, cl_index)):
        full = np.empty((B, NC_CLS), np.float32)
        for q in range(4):
            core = bi * 4 + q
            classes, ocols, halves = colmaps[core]
            oarr = np.asarray(res.results[core]["o"], np.float32)
            o3 = oarr.reshape(2, B, oarr.shape[1])
            full[:, classes] = o3[halves, :, ocols].T
        full += bs[bi][None, :]
        outs.append(full[:, np.asarray(index).astype(np.int64)])
    return outs[0], outs[1]


# revision 24
# speedup vs baseline: 1.0062x; 1.0036x over previous
"""GroupWiseLinear Trainium2 kernel (paired-stationary bins, ramped schedule).

out[b, c] = dot(W[0, c, :], x[b, group_of[c], :]) + bias[0, c], then a final
class-permutation gather, for two independent branches (co / cl).

Sharding: 8 cores = 2 branches x 4 shards.  Whole groups are assigned to
shards by LPT bin packing (~1024 classes each, no group straddles a core).
One uniform SPMD program runs on all cores; per-position bin widths are the
max over cores (zero-padded W elsewhere).

Device scheme per core:
  - Group segments (split at 200 cols) are paired so pair widths approach a
    target; each pair is a "bin": the two groups' x^T chunks side by side
    form a [128, 128] matmul stationary (batches 0-63 -> PSUM partitions
    0-63 for half A, 64-127 for half B).  The bin's W columns stream
    unpadded through one matmul per 128-deep K chunk (4), accumulating
    [128, N_bin] fp32 in PSUM.
  - Every output column is valid in exactly one half; the host picks the
    half, adds bias, and applies the final class permutation.
  - Schedule: bins are emitted smallest-first ramp -> big -> tiny tail.
    Each slice loads ONE combined [x-part | W-part] DMA (SP queue), its
    PSUM bank is copied to SBUF as bf16 (DVE) and output DMAs cover two
    slices each (9 DMA instructions total keeps the shared HWDGE
    descriptor generator off the critical path; tiny last slices keep the
    post-load tail short).

HBM traffic per core ~2.6 MB (bf16 wire, fp32 PSUM accumulate) vs ~4.5 MB
for the slot-padded baseline.
"""

import ml_dtypes
import numpy as np

import concourse.bacc as bacc
import concourse.tile as tile
from concourse import mybir
from concourse.bass_utils import run_bass_kernel_spmd

B = 64          # batch
H = 512         # hidden
NC_CLS = 4096   # classes per branch
KC = H // 128   # contraction chunks
PSUM_W = 512    # PSUM bank width in fp32 columns
MAX_SEG = 192   # split segments wider than this
PAIR_TGT = 152  # aim pair widths here

_cache = {}


def _segments(go):
    segs = []
    i = 0
    n = len(go)
    while i < n:
        g = int(go[i])
        j = i
        while j < n and go[j] == g:
            j += 1
        segs.append((g, i, j - i))
        i = j
    return segs


def _lpt_shard(segs, nshards):
    loads = [0] * nshards
    shards = [[] for _ in range(nshards)]
    for s in sorted(segs, key=lambda t: -t[2]):
        k = min(range(nshards), key=lambda i: loads[i])
        shards[k].append(s)
        loads[k] += s[2]
    return shards


def _make_bins(segs):
    """Split wide segments, then pair seeking PAIR_TGT total width."""
    items = []
    for g, s, w in segs:
        for o in range(0, w, MAX_SEG):
            items.append((g, s + o, min(MAX_SEG, w - o)))
    items.sort(key=lambda t: -t[2])
    bins = []
    while items:
        a = items.pop(0)
        if not items:
            bins.append([a])
            break
        want = PAIR_TGT - a[2]
        bi = min(range(len(items)), key=lambda i: abs(items[i][2] - want))
        if a[2] + items[bi][2] > PSUM_W:
            bins.append([a])
        else:
            bins.append([a, items.pop(bi)])
    bins.sort(key=lambda bn: -sum(p[2] for p in bn))
    return bins


def _plan(co_group_of, cl_group_of):
    gos = (np.asarray(co_group_of).astype(np.int64),
           np.asarray(cl_group_of).astype(np.int64))
    core_bins = []
    for bi in range(2):
        for sh in _lpt_shard(_segments(gos[bi]), 4):
            core_bins.append(_make_bins(sh))
    nbins = max(len(cb) for cb in core_bins)
    for cb in core_bins:
        while len(cb) < nbins:
            cb.append([])
    prof_desc = [
        max(sum(p[2] for p in cb[j]) for cb in core_bins) for j in range(nbins)
    ]
    prof_desc = [(max(w, 8) + 7) // 8 * 8 for w in prof_desc]

    # Emission order: one mid bin first (PE warm-up on a small load), then
    # descending sizes, tiny bins last.
    mid = min(nbins - 1, (nbins + 1) // 2)
    order = [mid] + [j for j in range(nbins) if j != mid]
    profile = [prof_desc[j] for j in order]
    for cb in core_bins:
        cb[:] = [cb[j] for j in order]

    # Slices: consecutive bins, each slice's width <= PSUM_W, sized so the
    # stream ramps: first slice 1 bin, then 2 bins each, tail 1-bin slices.
    sizes = [1]
    rem = nbins - 1
    while rem > 2:
        take = 2 if rem > 4 else 1
        sizes.append(take)
        rem -= take
    while rem:
        sizes.append(1)
        rem -= 1
    bounds = [0]
    for s in sizes:
        bounds.append(bounds[-1] + s)

    # PSUM banks: greedy <=PSUM_W within each slice.
    tiles = []
    for s in range(len(sizes)):
        b0, b1 = bounds[s], bounds[s + 1]
        t0, acc = b0, 0
        for j in range(b0, b1):
            if acc + profile[j] > PSUM_W:
                tiles.append((s, t0, j))
                t0, acc = j, 0
            acc += profile[j]
        tiles.append((s, t0, b1))

    # Last bin position is a lone item (or empty) on every core -> its
    # stationary only needs 64 columns (M=64), halving its xt block.
    single_last = all(len(cb[nbins - 1]) <= 1 for cb in core_bins)

    # Output DMA groups: pairs of consecutive slices, except one merged
    # group for the small tail slices so only ONE out-DMA chain trails the
    # final load.
    nsl = len(sizes)
    tail0 = nsl
    tailw = 0
    while tail0 > 1 and tailw + profile[bounds[tail0 - 1]] <= PSUM_W // 2:
        tailw += sum(profile[bounds[tail0 - 1] : bounds[tail0]])
        tail0 -= 1
    ogroups = []
    s = 0
    while s < tail0:
        s1 = min(s + 2, tail0)
        ogroups.append((s, s1))
        s = s1
    if tail0 < nsl:
        ogroups.append((tail0, nsl))
    return core_bins, profile, bounds, tiles, ogroups, single_last


def _xw_layout(profile, bounds, single_last):
    """Per-slice (offset, per-bin x offsets, x cols, w cols) in the combined
    xw buffer; the last bin's stationary is 64 wide when single_last."""
    nbins = len(profile)
    nsl = len(bounds) - 1
    prefn = np.concatenate([[0], np.cumsum(profile)]).astype(int)
    xblk = [KC * 128] * nbins
    if single_last:
        xblk[nbins - 1] = KC * 64
    soff = []
    off = 0
    for s in range(nsl):
        b0, b1 = bounds[s], bounds[s + 1]
        xoffs = []
        xc = 0
        for j in range(b0, b1):
            xoffs.append(xc)
            xc += xblk[j]
        wc = KC * int(prefn[b1] - prefn[b0])
        soff.append((off, xoffs, xc, wc))
        off += xc + wc
    return soff, off, prefn


def _program(profile, bounds, tiles, ogroups, single_last,
             dt=mybir.dt.bfloat16):
    nbins = len(profile)
    nsl = len(bounds) - 1
    soff, tot_xw, prefn = _xw_layout(profile, bounds, single_last)
    totn = int(prefn[-1])

    nc = bacc.Bacc("TRN2", target_bir_lowering=False, debug=False, num_devices=8)
    xw_d = nc.dram_tensor("xw", [128, tot_xw], dt, kind="ExternalInput")
    o_d = nc.dram_tensor("o", [128, totn], dt, kind="ExternalOutput")

    with tile.TileContext(nc) as tc:
        with (
            tc.tile_pool(name="xp", bufs=nsl) as xp,
            tc.tile_pool(name="op", bufs=1) as op,
            tc.tile_pool(name="ps", bufs=1, space="PSUM") as ps,
        ):
            stiles = {}
            for s in range(nsl):
                o0, _, xc, wc = soff[s]
                xws = xp.tile([128, xc + wc], dt)
                nc.sync.dma_start(xws[:], xw_d[:, o0 : o0 + xc + wc])
                stiles[s] = xws

            # one SBUF output tile per output group; PSUM copies land in it
            grp_of_slice = {}
            for gi, (g0, g1) in enumerate(ogroups):
                for s in range(g0, g1):
                    grp_of_slice[s] = gi
            gtiles = {}
            for gi, (g0, g1) in enumerate(ogroups):
                gw = int(prefn[bounds[g1]] - prefn[bounds[g0]])
                gtiles[gi] = op.tile([128, gw], dt, name=f"og{gi}")

            for ti, (s, tb0, tb1) in enumerate(tiles):
                b0, b1 = bounds[s], bounds[s + 1]
                o0, xoffs, xc, wc = soff[s]
                xws = stiles[s]
                tw = int(prefn[tb1] - prefn[tb0])
                acc = ps.tile([128, tw], mybir.dt.float32, name=f"ps{ti}")
                for j in range(tb0, tb1):
                    n = profile[j]
                    m = 64 if (single_last and j == nbins - 1) else 128
                    xoff = xoffs[j - b0]
                    woff = xc + KC * int(prefn[j] - prefn[b0])
                    aoff = int(prefn[j] - prefn[tb0])
                    for k in range(KC):
                        nc.tensor.matmul(
                            acc[0:m, aoff : aoff + n],
                            xws[:, xoff + k * m : xoff + (k + 1) * m],
                            xws[:, woff + k * n : woff + (k + 1) * n],
                            start=(k == 0),
                            stop=(k == KC - 1),
                        )
                gi = grp_of_slice[s]
                g0, g1 = ogroups[gi]
                got = gtiles[gi]
                goff = int(prefn[tb0] - prefn[bounds[g0]])
                nc.vector.tensor_copy(got[:, goff : goff + tw], acc[:])
                if tb1 == bounds[g1]:  # group complete -> single out DMA
                    # final group rides SP: shorter decode + DGE delay
                    eng = nc.sync if gi == len(ogroups) - 1 else nc.scalar
                    eng.dma_start(
                        o_d[:, int(prefn[bounds[g0]]) : int(prefn[bounds[g1]])],
                        got[:],
                    )

    nc.compile()
    return nc


def _host_prep(x, W, bins, profile, bounds, single_last, goff,
               dt=ml_dtypes.bfloat16):
    nbins = len(profile)
    soff, tot_xw, prefn = _xw_layout(profile, bounds, single_last)
    xw = np.zeros((128, tot_xw), dt)

    classes, ocols, halves = [], [], []
    xg_cache = {}

    def xgT(g):
        if g not in xg_cache:
            xg_cache[g] = np.ascontiguousarray(
                x[:, goff + g, :].astype(np.float32)
                .reshape(B, KC, 128).transpose(2, 1, 0)
            ).astype(dt)  # [128, KC, 64]
        return xg_cache[g]

    sl_of = np.searchsorted(bounds, np.arange(nbins), side="right") - 1
    for j, bn in enumerate(bins):
        s = int(sl_of[j])
        b0 = bounds[s]
        o0, xoffs, xc, wc = soff[s]
        n = profile[j]
        m = 64 if (single_last and j == nbins - 1) else 128
        coff = 0
        for hidx, (g, cs, w) in enumerate(bn):
            xs = xgT(g)
            for k in range(KC):
                c0 = o0 + xoffs[j - b0] + k * m + hidx * 64
                xw[:, c0 : c0 + 64] = xs[:, k, :]
            wTk = W[cs : cs + w].astype(np.float32).reshape(w, KC, 128)
            for k in range(KC):
                wcol = o0 + xc + KC * int(prefn[j] - prefn[b0]) + k * n + coff
                xw[:, wcol : wcol + w] = wTk[:, k, :].T
            classes.append(np.arange(cs, cs + w))
            ocols.append(int(prefn[j]) + coff + np.arange(w))
            halves.append(np.full(w, hidx))
            coff += w
    return ({"xw": xw},
            np.concatenate(classes), np.concatenate(ocols),
            np.concatenate(halves))


def kernel(x, co_W, cl_W, co_b, cl_b, co_group_of, cl_group_of, co_index,
           cl_index, group_len):
    x = np.asarray(x, np.float32)
    G = int(group_len)
    core_bins, profile, bounds, tiles, ogroups, single_last = _plan(
        co_group_of, cl_group_of
    )

    key = ("v5", tuple(profile), tuple(bounds), tuple(tiles), tuple(ogroups),
           single_last)
    if key not in _cache:
        _cache.clear()
        _cache[key] = _program(profile, bounds, tiles, ogroups, single_last)
    nc = _cache[key]

    Ws = (np.asarray(co_W, np.float32)[0], np.asarray(cl_W, np.float32)[0])
    bs = (np.asarray(co_b, np.float32)[0], np.asarray(cl_b, np.float32)[0])
    in_maps, colmaps = [], []
    for k in range(8):
        bi = k // 4
        im, classes, ocols, halves = _host_prep(
            x, Ws[bi], core_bins[k], profile, bounds, single_last, bi * G
        )
        in_maps.append(im)
        colmaps.append((classes, ocols, halves))

    res = run_bass_kernel_spmd(nc, in_maps, list(range(8)))

    outs = []
    for bi, index in ((0, co_index), (1, cl_index)):
        full = np.empty((B, NC_CLS), np.float32)
        for q in range(4):
            core = bi * 4 + q
            classes, ocols, halves = colmaps[core]
            oarr = np.asarray(res.results[core]["o"], np.float32)
            o3 = oarr.reshape(2, B, oarr.shape[1])
            full[:, classes] = o3[halves, :, ocols].T
        full += bs[bi][None, :]
        outs.append(full[:, np.asarray(index).astype(np.int64)])
    return outs[0], outs[1]
